# revision 1
# baseline (speedup 1.0000x reference)
"""PVT-style spatial-reduction attention on 8 TRN2 NeuronCores.

Sharding: core c -> (batch b = c//2, head-group g = c%2), 4 heads each.
No collectives: each core computes a partial projection output
outT_partial (512, 4096); host sums the two partials per batch.

On-core orientation: features-on-partition, tokens-on-free throughout:
  xT (ch, tok) -> convT (oc, pos) -> LN -> kT (kc, pos) / v (pos, vc)
  qT (qc, tok); scoresT (kv, tok) = kT_h^T-slice @ qT_h; exp on ACT;
  avT (65, tok) via v4 lhsT with ones column (row 64 = softmax denom);
  netT = avT * bcast(1/denom); outT = proj_w_g^T-slice @ netT.
All matmuls bf16 (f32 PSUM) except tiny f32 broadcast matmuls.
"""
import sys as _sys
for _p in ("/opt/trn_rl_repo", "/opt/pypackages"):
    if _p not in _sys.path:
        _sys.path.insert(0, _p)

import numpy as np
import ml_dtypes
from contextlib import ExitStack

import concourse.bass as bass
import concourse.mybir as mybir
import concourse.tile as tile
from concourse import bacc
from concourse.bass_utils import run_bass_kernel_spmd

BF = mybir.dt.bfloat16
F32 = mybir.dt.float32
P = 128
BS, N, DIM, HEADS, HD = 4, 4096, 512, 8, 64
NKV = 1024
SCALE = HD ** -0.5  # 0.125
EPS = 1e-5
NCH = 8          # token chunks of 512
CHUNK = N // NCH  # 512
NBF = np.dtype(ml_dtypes.bfloat16)


DEBUG = False


def build_nc():
    nc = bacc.Bacc()
    xt_d = nc.declare_dram_parameter("xt", (4, P, N), BF, isOutput=False)
    wc_d = nc.declare_dram_parameter("wc", (16, P, DIM), BF, isOutput=False)
    qw_d = nc.declare_dram_parameter("qw", (4, P, 256), BF, isOutput=False)
    kw_d = nc.declare_dram_parameter("kw", (4, P, 256), BF, isOutput=False)
    vw_d = nc.declare_dram_parameter("vw", (4, P, 256), BF, isOutput=False)
    pw_d = nc.declare_dram_parameter("pw", (2, P, DIM), BF, isOutput=False)
    b4_d = nc.declare_dram_parameter("b4", (P, 4), F32, isOutput=False)
    kb_d = nc.declare_dram_parameter("kb", (P, 2), F32, isOutput=False)
    vb_d = nc.declare_dram_parameter("vb", (1, 256), BF, isOutput=False)
    out_d = nc.declare_dram_parameter("out", (DIM, N), F32, isOutput=True)
    if DEBUG:
        dbg_xrn = nc.declare_dram_parameter("dbg_xrn", (P, 4, NKV), F32, isOutput=True)
        dbg_kT2 = nc.declare_dram_parameter("dbg_kT2", (64, 4, NKV), F32, isOutput=True)
        dbg_qT2 = nc.declare_dram_parameter("dbg_qT2", (64, 4, N), F32, isOutput=True)
        dbg_v4 = nc.declare_dram_parameter("dbg_v4", (P, 8, 4, 65), F32, isOutput=True)
        dbg_net = nc.declare_dram_parameter("dbg_net", (P, 2, N), F32, isOutput=True)
        dbg_av = nc.declare_dram_parameter("dbg_av", (P, CHUNK), F32, isOutput=True)
        dbg_avu = nc.declare_dram_parameter("dbg_avu", (P, CHUNK), F32, isOutput=True)
        dbg_rb = nc.declare_dram_parameter("dbg_rb", (P, CHUNK), F32, isOutput=True)
        dbg_e = nc.declare_dram_parameter("dbg_e", (P, 4, CHUNK), F32, isOutput=True)

    with tile.TileContext(nc) as tc, ExitStack() as ctx:
        persist = ctx.enter_context(tc.tile_pool(name="persist", bufs=1))

        # ---- persistent SBUF tensors
        xt = persist.tile([P, 4, N], BF, tag="xt")
        wc = persist.tile([P, 16, DIM], BF, tag="wc")
        qw = persist.tile([P, 4, 256], BF, tag="qw")
        kw = persist.tile([P, 4, 256], BF, tag="kw")
        vw = persist.tile([P, 4, 256], BF, tag="vw")
        pw = persist.tile([P, 2, DIM], BF, tag="pw")
        b4 = persist.tile([P, 4], F32, tag="b4")
        kb = persist.tile([P, 2], F32, tag="kb")
        vb = persist.tile([1, 256], BF, tag="vb")

        xrc = persist.tile([P, 4, NKV], BF, tag="xrc")    # centered conv out
        xrn = persist.tile([P, 4, NKV], BF, tag="xrn")    # LN'd
        qT2 = persist.tile([64, 4, N], BF, tag="qT2")     # per-head q rows
        kT2 = persist.tile([64, 4, NKV], BF, tag="kT2")   # per-head k rows
        v4 = persist.tile([P, 8, 4, 128], BF, tag="v4")   # [ones|pad|v]
        netT = persist.tile([P, 2, N], BF, tag="netT")
        rstd = persist.tile([1, NKV], F32, tag="rstd")
        stdt = persist.tile([1, NKV], F32, tag="stdt")

        ones_inv = persist.tile([P, 1], BF, tag="ones_inv")   # 1/512 column (K=128, M=1)
        ones128f = persist.tile([1, P], F32, tag="ones128f")  # f32 ones row (K=1, M=128)
        eps1 = persist.tile([1, 1], F32, tag="eps1")

        # ---- DMAs: qw first (q fills conv's DMA stalls), xt in quarters,
        # conv weights, then later-stage weights
        nc.sync.dma_start(b4[:], b4_d[:])
        nc.sync.dma_start(kb[:], kb_d[:])
        nc.sync.dma_start(vb[:], vb_d[:])
        for kt in range(4):
            nc.sync.dma_start(qw[:, kt, :], qw_d[kt])
            nc.sync.dma_start(xt[:, kt, 0:1024], xt_d[kt][:, 0:1024])
        for kt in range(4):
            nc.sync.dma_start(xt[:, kt, 1024:2048], xt_d[kt][:, 1024:2048])
        for kt in range(4):
            for d in range(4):
                nc.sync.dma_start(wc[:, d * 4 + kt, :], wc_d[d * 4 + kt])
        for q4 in range(2, 4):
            for kt in range(4):
                nc.sync.dma_start(xt[:, kt, q4 * 1024:(q4 + 1) * 1024],
                                  xt_d[kt][:, q4 * 1024:(q4 + 1) * 1024])
        for kt in range(4):
            nc.sync.dma_start(kw[:, kt, :], kw_d[kt])
            nc.sync.dma_start(vw[:, kt, :], vw_d[kt])
        nc.sync.dma_start(pw[:, 0, :], pw_d[0])
        nc.sync.dma_start(pw[:, 1, :], pw_d[1])

        nc.vector.memset(ones_inv[:], 1.0 / DIM)
        nc.vector.memset(ones128f[:], 1.0)
        nc.vector.memset(v4[:], 0.0)
        nc.vector.memset(v4[:, :, :, 0:1], 1.0)
        nc.vector.memset(eps1[:], EPS)
        vbb = persist.tile([P, 256], BF, tag="vbb")
        nc.gpsimd.partition_broadcast(vbb[:], vb[:])

        esb = ctx.enter_context(tc.tile_pool(name="esb", bufs=12))
        dbgp = ctx.enter_context(tc.tile_pool(name="dbgp", bufs=1))
        recdp = ctx.enter_context(tc.tile_pool(name="recdp", bufs=3))
        avup = ctx.enter_context(tc.tile_pool(name="avup", bufs=3))
        osbp = ctx.enter_context(tc.tile_pool(name="osbp", bufs=4))
        rbbp = ctx.enter_context(tc.tile_pool(name="rbb", bufs=4))
        ph1 = ExitStack()
        wkps = ph1.enter_context(tc.tile_pool(name="work", bufs=4, space="PSUM"))
        avps1 = ph1.enter_context(tc.tile_pool(name="avrb1", bufs=2, space="PSUM"))
        p1sb = ph1.enter_context(tc.tile_pool(name="p1sb", bufs=4))
        if True:

            def emit_conv(n):
                vt = avps1.tile([1, 512], F32, tag="av")
                for oct_ in range(4):  # oc tiles
                    cps = wkps.tile([P, 512], F32, tag="wk")
                    first = True
                    for kt in range(4):       # kt-major: matches DMA arrival
                        for d in range(4):
                            w = d * 4 + kt
                            di, dj = d // 2, d % 2
                            xv = xt[:, kt, :].rearrange(
                                "p (i a j b) -> p i a j b", i=32, a=2, j=32, b=2)
                            nc.tensor.matmul(
                                cps[:],
                                wc[:, w, oct_ * P:(oct_ + 1) * P],
                                xv[:, 16 * n:16 * (n + 1), di, :, dj],
                                start=first, stop=(kt == 3 and d == 3),
                            )
                            first = False
                    nc.scalar.activation(
                        xrc[:, oct_, n * 512:(n + 1) * 512], cps[:],
                        mybir.ActivationFunctionType.Identity,
                        bias=b4[:, oct_:oct_ + 1])
                    sq = p1sb.tile([P, 512], BF, tag="sq")
                    nc.scalar.activation(
                        sq[:], cps[:],
                        mybir.ActivationFunctionType.Square,
                        bias=b4[:, oct_:oct_ + 1])
                    nc.tensor.matmul(
                        vt[:], ones_inv[:],
                        sq[:], start=(oct_ == 0), stop=(oct_ == 3),
                    )
                return vt

            def emit_ln(n, vt):
                ns = slice(n * 512, (n + 1) * 512)
                # rstd = exp(-0.5*ln(var+eps)) — single ACT table set
                nc.scalar.activation(
                    stdt[0:1, ns], vt[:],
                    mybir.ActivationFunctionType.Ln, bias=eps1[0:1, 0:1])
                nc.scalar.activation(rstd[0:1, ns], stdt[0:1, ns],
                                     mybir.ActivationFunctionType.Exp, scale=-0.5)
                rbc = avps1.tile([P, 512], F32, tag="av")
                nc.tensor.matmul(rbc[:], ones128f[:], rstd[0:1, ns],
                                 start=True, stop=True)
                for kt in range(4):
                    nc.vector.tensor_tensor(
                        xrn[:, kt, ns], xrc[:, kt, ns], rbc[:],
                        mybir.AluOpType.mult)

            def emit_k(n):
                ns = slice(n * 512, (n + 1) * 512)
                for m in range(2):
                    kps = wkps.tile([P, 512], F32, tag="wk")
                    for kt in range(4):
                        nc.tensor.matmul(
                            kps[:],
                            kw[:, kt, m * P:(m + 1) * P],
                            xrn[:, kt, ns],
                            start=(kt == 0), stop=(kt == 3),
                        )
                    nc.scalar.activation(
                        kT2[0:64, 2 * m, ns], kps[0:64, :],
                        mybir.ActivationFunctionType.Identity,
                        bias=kb[0:64, m:m + 1])
                    nc.scalar.activation(
                        kT2[0:64, 2 * m + 1, ns], kps[64:128, :],
                        mybir.ActivationFunctionType.Identity,
                        bias=kb[64:128, m:m + 1])

            def emit_v(n):
                for pt in range(4 * n, 4 * (n + 1)):
                    vps = prps.tile([P, 256], F32, tag="pr")
                    for kt in range(4):
                        nc.tensor.matmul(
                            vps[:],
                            xrn[:, kt, pt * P:(pt + 1) * P],
                            vw[:, kt, :],
                            start=(kt == 0), stop=(kt == 3),
                        )
                    nc.vector.scalar_tensor_tensor(
                        v4[:, pt, :, 64:128],
                        vps[:].rearrange("p (h d) -> p h d", h=4),
                        0.0,
                        vbb[:].rearrange("p (h d) -> p h d", h=4),
                        mybir.AluOpType.add,
                        mybir.AluOpType.add)

            def emit_q(c):
                cs = slice(c * CHUNK, (c + 1) * CHUNK)
                for m in range(2):
                    qps = qpool[0].tile([P, CHUNK], F32, tag=qpool[1])
                    for kt in range(4):
                        nc.tensor.matmul(
                            qps[:],
                            qw[:, kt, m * P:(m + 1) * P],
                            xt[:, kt, cs],
                            start=(kt == 0), stop=(kt == 3),
                        )
                    nc.vector.tensor_copy(qT2[0:64, 2 * m, cs], qps[0:64, :])
                    nc.vector.tensor_copy(qT2[0:64, 2 * m + 1, cs], qps[64:128, :])

            def emit_proj(pc):
                pcs = slice(pc * CHUNK, (pc + 1) * CHUNK)
                for oct_ in range(4):
                    pps = prps.tile([P, CHUNK], F32, tag="pr")
                    for kt in range(2):
                        nc.tensor.matmul(
                            pps[:],
                            pw[:, kt, oct_ * P:(oct_ + 1) * P],
                            netT[:, kt, pcs],
                            start=(kt == 0), stop=(kt == 1),
                        )
                    osb = osbp.tile([P, CHUNK], F32, tag="osb")
                    nc.vector.tensor_copy(osb[:], pps[:])
                    nc.sync.dma_start(out_d[oct_ * P:(oct_ + 1) * P, pcs], osb[:])

            # ---- software-pipelined attention: one stream of 32 (c, h)
            # tasks; scores+exp of task i overlap av/normalize of task i-1.
            state = {}

            def emit_scores(i):
                c, h = i // 4, i % 4
                cs = slice(c * CHUNK, (c + 1) * CHUNK)
                etiles = []
                for grp in range(4):
                    sps = spsp.tile([P, 2, CHUNK], F32, tag="s")
                    for ti in range(2):
                        t = grp * 2 + ti
                        nc.tensor.matmul(
                            sps[:, ti, :],
                            kT2[0:64, h, t * P:(t + 1) * P],
                            qT2[0:64, h, cs],
                            start=True, stop=True,
                        )
                    ebf = esb.tile([P, 2, CHUNK], BF, tag="e")
                    nc.scalar.activation(
                        ebf[:], sps[:],
                        mybir.ActivationFunctionType.Exp, scale=SCALE)
                    etiles.append(ebf)
                state[i] = etiles

            def emit_av(i):
                c, h = i // 4, i % 4
                cs = slice(c * CHUNK, (c + 1) * CHUNK)
                etiles = state.pop(i)
                avt = avps.tile([P, CHUNK], F32, tag="av")
                for t in range(8):
                    nc.tensor.matmul(
                        avt[0:128, :],
                        v4[:, t, h, :],
                        etiles[t // 2][:, t % 2, :],
                        start=(t == 0), stop=(t == 7),
                    )
                if DEBUG and c == 0 and h == 0:
                    dbga = dbgp.tile([P, CHUNK], F32, tag="dbgt")
                    nc.vector.tensor_copy(dbga[0:128, :], avt[0:128, :])
                    nc.sync.dma_start(dbg_av[0:128, :], dbga[0:128, :])
                    dbge = dbgp.tile([P, 4, CHUNK], F32, tag="dbgt")
                    nc.vector.tensor_copy(dbge[:, 0:2, :], etiles[0][:])
                    nc.vector.tensor_copy(dbge[:, 2:4, :], etiles[1][:])
                    nc.sync.dma_start(dbg_e[:], dbge[:])
                recf = recdp.tile([1, CHUNK], F32, tag="recf")
                nc.vector.reciprocal_approx_fast(out=recf[:], in_=avt[0:1, :])
                # broadcast 1/denom across partitions on the idle GPSIMD engine
                rbb = rbbp.tile([P, CHUNK], F32, tag=("rbe" if h % 2 == 0 else "rbo"))
                nc.gpsimd.partition_broadcast(rbb[:], recf[:])
                state[("rbb", i)] = rbb
                if h % 2 == 0:
                    avu_pair = avup.tile([P, CHUNK], BF, tag="avu")
                    nc.vector.tensor_copy(avu_pair[0:64, :], avt[64:128, :])
                    state[("avu", i)] = avu_pair
                else:
                    avu_pair = state.pop(("avu", i - 1))
                    nc.vector.tensor_copy(avu_pair[64:128, :], avt[64:128, :])
                    pr = h // 2
                    rbb_e = state.pop(("rbb", i - 1))
                    rbb_o = state.pop(("rbb", i))
                    if DEBUG and c == 0 and h == 1:
                        dbgu = dbgp.tile([P, CHUNK], F32, tag="dbgt")
                        nc.vector.tensor_copy(dbgu[:], avu_pair[:])
                        nc.sync.dma_start(dbg_avu[:], dbgu[:])
                        dbgr = dbgp.tile([P, CHUNK], F32, tag="dbgt")
                        nc.vector.tensor_copy(dbgr[0:64, :], rbb_e[0:64, :])
                        nc.vector.tensor_copy(dbgr[64:128, :], rbb_o[64:128, :])
                        nc.sync.dma_start(dbg_rb[:], dbgr[:])
                    nc.vector.tensor_tensor(
                        netT[0:64, pr, cs], avu_pair[0:64, :], rbb_e[0:64, :],
                        mybir.AluOpType.mult)
                    nc.vector.tensor_tensor(
                        netT[64:128, pr, cs], avu_pair[64:128, :], rbb_o[64:128, :],
                        mybir.AluOpType.mult)

            # ---- emission: conv, q(c0,c1) fill DMA stalls, LN, kv, then the
            # pipelined task stream with q one chunk ahead
            qpool = (wkps, "wk")
            vt0 = emit_conv(0)
            emit_q(0)
            emit_q(1)
            vt1 = emit_conv(1)
            emit_ln(0, vt0)
            emit_ln(1, vt1)
            emit_k(0)
            emit_k(1)
            ph1.close()
            ph2 = ExitStack()
            spsp = ph2.enter_context(tc.tile_pool(name="sps", bufs=2, space="PSUM"))
            avps = ph2.enter_context(tc.tile_pool(name="avrb", bufs=2, space="PSUM"))
            prps = ph2.enter_context(tc.tile_pool(name="prj", bufs=2, space="PSUM"))
            qpool = (prps, "pr")
            NT = 4 * NCH
            for i in range(NT + 1):
                if i < NT:
                    if i % 4 == 0 and (i // 4) + 2 <= NCH - 1:
                        emit_q((i // 4) + 2)
                    emit_scores(i)
                if i == 0:
                    emit_v(0)
                    emit_v(1)
                if i >= 1:
                    emit_av(i - 1)
                    if (i - 1) % 4 == 3:
                        emit_proj((i - 1) // 4)
            ph2.close()

            if DEBUG:
                dbg_pool = tc.tile_pool(name="dbg", bufs=1)
                with dbg_pool as dp:
                    def dump(dram, sb_ap, nparts):
                        total = 1
                        for s in sb_ap.shape[1:]:
                            total *= s
                        pat_in = {3: "p a b -> p (a b)", 4: "p a b c -> p (a b c)"}
                        flat = (sb_ap.rearrange(pat_in[len(sb_ap.shape)])
                                if len(sb_ap.shape) > 2 else sb_ap)
                        dflat = (dram.rearrange(pat_in[len(dram.shape)])
                                 if len(dram.shape) > 2 else dram)
                        for off in range(0, total, 512):
                            w = min(512, total - off)
                            tt = dp.tile([nparts, 512], F32, tag="dbgt")
                            nc.vector.tensor_copy(tt[:, :w], flat[:, off:off + w])
                            nc.sync.dma_start(dflat[:, off:off + w], tt[:, :w])
                    dump(dbg_xrn, xrn[:], P)
                    dump(dbg_kT2, kT2[:], 64)
                    dump(dbg_qT2, qT2[:], 64)
                    dump(dbg_v4, v4[:], P)
                    dump(dbg_net, netT[:], P)

    nc.finalize()
    return nc


_NC_CACHE = {}


def _get_nc():
    if "nc" not in _NC_CACHE:
        _NC_CACHE["nc"] = build_nc()
    return _NC_CACHE["nc"]


def _prep_core_inputs(x, q_w, kv_w, proj_w, sr_w, sr_b, ln_w, ln_b):
    """Host-side sharding/prep. Returns list of 8 in_maps."""
    bf = NBF
    # conv weights: W4[d, ic, oc] = sr_w[oc, ic, di, dj], d = di*2+dj
    W4 = np.ascontiguousarray(sr_w.transpose(2, 3, 1, 0)).reshape(4, DIM, DIM)
    W4 = W4 - W4.mean(axis=2, keepdims=True)
    wc = W4.reshape(4, 4, P, DIM).reshape(16, P, DIM).astype(bf)
    b4 = (sr_b - sr_b.mean()).reshape(4, P).T.astype(np.float32)
    b4 = np.ascontiguousarray(b4)

    # token gather indices for the strided conv
    ii, jj = np.meshgrid(np.arange(32), np.arange(32), indexing="ij")
    toks = {}
    for di in range(2):
        for dj in range(2):
            toks[di * 2 + dj] = ((2 * ii + di) * 64 + (2 * jj + dj)).reshape(-1)

    kv_w_f = ln_w[:, None] * kv_w          # fold ln weight
    kv_bias = ln_b @ kv_w                   # fold ln bias (1024,)

    in_maps = []
    for c in range(8):
        b, g = c // 2, c % 2
        xT = np.ascontiguousarray(x[b].T)               # (512, 4096)
        xt = xT.reshape(4, P, N).astype(bf)
        qw = np.ascontiguousarray(
            q_w[:, g * 256:(g + 1) * 256]).reshape(4, P, 256).astype(bf)
        kw = np.ascontiguousarray(
            kv_w_f[:, g * 256:(g + 1) * 256]).reshape(4, P, 256).astype(bf)
        vw = np.ascontiguousarray(
            kv_w_f[:, DIM + g * 256:DIM + (g + 1) * 256]).reshape(4, P, 256).astype(bf)
        pwv = np.ascontiguousarray(
            proj_w[g * 256:(g + 1) * 256, :]).reshape(2, P, DIM).astype(bf)
        kbv = np.ascontiguousarray(
            kv_bias[g * 256:(g + 1) * 256].reshape(2, P).T).astype(np.float32)
        vbv = kv_bias[DIM + g * 256:DIM + (g + 1) * 256].reshape(1, 256).astype(bf)
        in_maps.append({
            "xt": xt, "wc": wc, "qw": qw, "kw": kw, "vw": vw,
            "pw": pwv, "b4": b4, "kb": kbv, "vb": vbv,
        })
    return in_maps


def kernel(x, q_w, kv_w, proj_w, proj_b, sr_w, sr_b, ln_w, ln_b, H, W,
           _return_perf=False):
    x = np.asarray(x, dtype=np.float32)
    q_w = np.asarray(q_w, dtype=np.float32)
    kv_w = np.asarray(kv_w, dtype=np.float32)
    proj_w = np.asarray(proj_w, dtype=np.float32)
    proj_b = np.asarray(proj_b, dtype=np.float32)
    sr_w = np.asarray(sr_w, dtype=np.float32)
    sr_b = np.asarray(sr_b, dtype=np.float32)
    ln_w = np.asarray(ln_w, dtype=np.float32)
    ln_b = np.asarray(ln_b, dtype=np.float32)

    in_maps = _prep_core_inputs(x, q_w, kv_w, proj_w, sr_w, sr_b, ln_w, ln_b)
    nc = _get_nc()
    res = run_bass_kernel_spmd(nc, in_maps, core_ids=list(range(8)),
                               trace=_return_perf)
    out = np.empty((BS, N, DIM), dtype=np.float32)
    for b in range(BS):
        partial = res.results[2 * b]["out"] + res.results[2 * b + 1]["out"]
        out[b] = partial.T + proj_b[None, :]
    if _return_perf:
        return out, res
    return out



# revision 90
# speedup vs baseline: 1.1701x; 1.1701x over previous
"""PVT-style spatial-reduction attention on 8 TRN2 NeuronCores.

Sharding: core c -> (batch b = c//2, head-group g = c%2), 4 heads each.
No collectives: each core computes outT_partial (512, 4096) in bf16;
host sums the two partials per batch and un-permutes tokens.

Structure (191.2us cost-model time vs 208.5us bf16 baseline):
 - Tokens host-permuted to tap-major order (im2col for the stride-2
   conv is a pure permutation), so conv rhs reads are contiguous.
 - conv and q projection run as fp8e4 DoubleRow matmuls (0.5 c/row,
   2 k-tiles/instr) with hi/lo operand splits for precision; fp8
   tensors are pre-scaled by powers of 2 (SX/SWC/SQW/SK) to stay in
   e4m3's normal range, compensated via ACT scale params.
 - q@k scores: DoubleRow with k_hi in pair-0, k's fp8 storage residual
   k_lo in pair-1, and a stride-0 broadcast feeding q to both pairs,
   so one instr computes (k_hi+k_lo).T @ q at K=64.
 - exp(softmax) split between ACT (true exp) and a custom DVE op
   (EXP8: (quadratic)^8 minimax fit of exp(SCALE/SK*x), ~0.3% err).
 - av is token-partition-major: out(tok,65)=[denom|e@v] via e-as-lhsT
   (no wasted output partitions), normalized by a reciprocal broadcast
   multiply, then PE-transposed back to feature-major for the proj.
   s-outer accumulation order (t-outer corrupts PSUM pending-zero).
 - Schedule: q first (DMA-gated), conv0/ln0/k0, early half-scores for
   the first W tasks interleaved with conv1/k1/v1, then a software-
   pipelined 32-task stream (scores -> av_head -> av_tail -> proj-oct
   spread one per iteration). DMAs are merged into few large transfers
   (each dma_start costs ~625ns of serial descriptor generation).
"""
import sys as _sys
for _p in ("/opt/trn_rl_repo", "/opt/pypackages"):
    if _p not in _sys.path:
        _sys.path.insert(0, _p)

import numpy as np
import ml_dtypes
from contextlib import ExitStack

import concourse.bass as bass
import concourse.mybir as mybir
import concourse.tile as tile
from concourse import bacc
from concourse.bass_utils import run_bass_kernel_spmd
from concourse.masks import make_identity

from concourse.dve_ops import (OPS, CUSTOM_DVE_SPECS, _SUB_OPCODE_FOR_NAME,
                               DveOp)
from concourse.dve_spec import Spec, Src0, C0, C1, C2, sq as dve_sq, lower
from concourse.dve_uop import DveOpSpec
from concourse.dve_table_gen import dve_ver_for

BF = mybir.dt.bfloat16
F32 = mybir.dt.float32
FP8 = mybir.dt.float8e4
NF8 = ml_dtypes.float8_e4m3
NBF = np.dtype(ml_dtypes.bfloat16)
DRM = mybir.MatmulPerfMode.DoubleRow

P = 128
BS, N, DIM, HEADS, HD = 4, 4096, 512, 8, 64
NKV = 1024
SCALE = HD ** -0.5  # 0.125
EPS = 1e-5
CHUNK = 512
NCH = 8
# fp8 pre-scales (powers of 2) keeping values in e4m3's normal range;
# compensated via ACT activation scale params and the exp argument scale.
SX = 4.0     # x
SWC = 32.0   # conv weights
SQW = 32.0   # q weights
SK = 8.0     # k storage (scores come out as 8*q.k)

# ---------------- custom EXP8 DVE op: exp(SCALE*x) ~= q(x)^8 ----------------


def _fit_exp8_coeffs(scale: float, xmax: float = 16.0):
    """q(x) = c0 x^2 + c1 x + c2 with (q(x))^8 ~= exp(scale*x) for raw
    scores x in [-xmax, xmax]."""
    x = np.linspace(-xmax, xmax, 8001)
    t = np.exp(scale * x / 8.0)
    w = 1.0 / t
    A = np.stack([x * x, x, np.ones_like(x)], axis=1) * w[:, None]
    coef, *_ = np.linalg.lstsq(A, t * w, rcond=None)
    return float(coef[0]), float(coef[1]), float(coef[2])


EXP8_C = _fit_exp8_coeffs(SCALE / SK, xmax=14.0 * SK)


def _register_exp8():
    name = "EXP8_ANT"
    if name in _SUB_OPCODE_FOR_NAME:
        return next(op for op in OPS if op.name == name)
    spec = Spec(
        body=dve_sq(dve_sq(dve_sq((Src0 * C0 + C1) * Src0 + C2))),
        reference=lambda in0, in1, s0, s1, imm2: (
            (((in0 * s0 + s1) * in0 + imm2).astype(np.float32) ** 2) ** 2) ** 2,
    )
    row = max(_SUB_OPCODE_FOR_NAME.values()) + 1
    _SUB_OPCODE_FOR_NAME[name] = row
    ver = dve_ver_for("TRN2")
    spec_c = DveOpSpec(name=name, opcode=row, uops=lower(spec, ver=ver),
                       rd1_en=False)
    op = DveOp(name, spec, subdim=False, uops_sha={ver: spec_c.sha(ver)})
    OPS.append(op)
    CUSTOM_DVE_SPECS[name] = spec
    return op


EXP8 = _register_exp8()

EXP_DVE_EXTRA = int(__import__("os").environ.get("K_EXP_DVE_EXTRA", "3"))


def _exp_on_dve(i, j):
    """Which exp instructions run on DVE (custom op) vs ACT."""
    if j == 3:
        return True
    return j == 1 and i % 4 < EXP_DVE_EXTRA


import os as _os
DUMMY_WARM = int(_os.environ.get("K_DUMMY_WARM", "0"))
DUMMY_SC = int(_os.environ.get("K_DUMMY_SC", "0"))
DUMMY_AV = int(_os.environ.get("K_DUMMY_AV", "0"))
AV_TOUTER = int(_os.environ.get("K_AV_TOUTER", "0"))
AV_FIRST = int(_os.environ.get("K_AV_FIRST", "1"))
XRN_DVE = int(_os.environ.get("K_XRN_DVE", "1"))
QCOPY_ACT = int(_os.environ.get("K_QCOPY_ACT", "0"))
Q_ILV = int(_os.environ.get("K_Q_ILV", "1"))
SPS1 = int(_os.environ.get("K_SPS1", "0"))
ESB_BUFS = int(_os.environ.get("K_ESB_BUFS", "10"))
DEBUG = int(_os.environ.get("K_DEBUG", "0"))


def build_nc():
    nc = bacc.Bacc()
    xh8_d = nc.declare_dram_parameter("xh8", (4, P, 4, NKV), FP8, isOutput=False)
    xl8_d = nc.declare_dram_parameter("xl8", (4, P, 4, NKV), FP8, isOutput=False)
    wch_d = nc.declare_dram_parameter("wch", (16, P, DIM), FP8, isOutput=False)
    wcl_d = nc.declare_dram_parameter("wcl", (16, P, DIM), FP8, isOutput=False)
    qw_d = nc.declare_dram_parameter("qw", (P, 2, 2, 256), FP8, isOutput=False)
    ql_d = nc.declare_dram_parameter("ql", (P, 2, 2, 256), FP8, isOutput=False)
    kw_d = nc.declare_dram_parameter("kw", (4, P, 256), BF, isOutput=False)
    vw_d = nc.declare_dram_parameter("vw", (4, P, 256), BF, isOutput=False)
    pw_d = nc.declare_dram_parameter("pw", (2, P, DIM), BF, isOutput=False)
    b4_d = nc.declare_dram_parameter("b4", (P, 4), F32, isOutput=False)
    kbr_d = nc.declare_dram_parameter("kbr", (1, 256), BF, isOutput=False)
    vb_d = nc.declare_dram_parameter("vb", (1, 256), BF, isOutput=False)
    out_d = nc.declare_dram_parameter("out", (DIM, N), BF, isOutput=True)
    if DEBUG:
        dbg = {
            "xrn": nc.declare_dram_parameter("dbg_xrn", (P, 4, NKV), F32,
                                             isOutput=True),
            "qT8": nc.declare_dram_parameter("dbg_qT8", (P, 2, N), F32,
                                             isOutput=True),
            "kT8": nc.declare_dram_parameter("dbg_kT8", (P, 2, 2, NKV), F32,
                                             isOutput=True),
            "v4": nc.declare_dram_parameter("dbg_v4", (P, 8, 4, 65), F32,
                                            isOutput=True),
            "netT": nc.declare_dram_parameter("dbg_netT", (P, 2, N), F32,
                                              isOutput=True),
        }

    with tile.TileContext(nc) as tc, ExitStack() as ctx:
        persist = ctx.enter_context(tc.tile_pool(name="persist", bufs=1))

        xh8 = persist.tile([P, 4, 4, NKV], FP8, tag="xh8")
        xl8 = persist.tile([P, 4, 4, NKV], FP8, tag="xl8")
        wch = persist.tile([P, 16, DIM], FP8, tag="wch")
        wcl = persist.tile([P, 16, DIM], FP8, tag="wcl")
        qw8 = persist.tile([P, 2, 2, 256], FP8, tag="qw8")
        ql8 = persist.tile([P, 2, 2, 256], FP8, tag="ql8")
        kw = persist.tile([P, 4, 256], BF, tag="kw")
        kbr = persist.tile([1, 256], BF, tag="kbr")
        ones1 = persist.tile([1, 512], BF, tag="ones1")
        vw = persist.tile([P, 4, 256], BF, tag="vw")
        pw = persist.tile([P, 2, DIM], BF, tag="pw")
        b4 = persist.tile([P, 4], F32, tag="b4")
        vb = persist.tile([1, 256], BF, tag="vb")

        xrc = persist.tile([P, 4, NKV], BF, tag="xrc")
        xrn = persist.tile([P, 4, NKV], BF, tag="xrn")
        # partition halves hold even/odd heads: one 128-part copy
        # fills both heads of a pair (copy cost is free-size only)
        qT8 = persist.tile([P, 2, N], FP8, tag="qT8")
        kT8 = persist.tile([P, 2, 2, NKV], FP8, tag="kT8")
        v4 = persist.tile([P, 8, 4, 65], BF, tag="v4")
        netT = persist.tile([P, 2, N], BF, tag="netT")
        stdt = persist.tile([1, NKV], F32, tag="stdt")
        rstd = persist.tile([1, NKV], F32, tag="rstd")
        ident = persist.tile([P, P], BF, tag="ident")
        ones_inv = persist.tile([P, 1], BF, tag="ones_inv")
        eps1 = persist.tile([1, 1], F32, tag="eps1")
        vbb = persist.tile([P, 256], BF, tag="vbb")

        # ---- DMAs (qw8+xh8 first so q can start early; wc interleaved with
        # xtc by ctile so conv0's accumulation chain starts on first arrivals)
        nc.sync.dma_start(b4[:], b4_d[:])
        nc.sync.dma_start(kbr[:], kbr_d[:])
        nc.sync.dma_start(vb[:], vb_d[:])
        nc.sync.dma_start(qw8[:], qw_d[:])
        nc.sync.dma_start(ql8[:], ql_d[:])
        for c in range(4):
            nc.sync.dma_start(xh8[:, c, :, :], xh8_d[c])
        nc.sync.dma_start(
            wch[:], wch_d[:].rearrange("k p n -> p k n"))
        nc.sync.dma_start(
            wcl[:], wcl_d[:].rearrange("k p n -> p k n"))
        for c in range(4):
            nc.sync.dma_start(xl8[:, c, :, :], xl8_d[c])
        nc.sync.dma_start(kw[:], kw_d[:].rearrange("k p n -> p k n"))
        nc.sync.dma_start(vw[:], vw_d[:].rearrange("k p n -> p k n"))
        nc.sync.dma_start(pw[:], pw_d[:].rearrange("k p n -> p k n"))

        # pre-load the one ACT table covering Ln/Exp/Identity/Square so the
        # fixpoint pass doesn't insert per-switch table loads (1.3us each)
        from concourse.hw_specs import get_activation_tables
        _tset = list(get_activation_tables(nc.m.arch))
        nc.scalar.add_instruction(mybir.InstLoadActFuncSet(
            name=nc.get_next_instruction_name(), ins=[], outs=[],
            act_func_set_id=_tset.index("natural_log_exp_and_others")))

        # ---- init constants (Pool = gpsimd, SBUF only)
        nc.vector.memset(ones_inv[:], 1.0 / DIM)
        nc.vector.memset(ones1[:], 1.0)
        nc.vector.memset(eps1[:], EPS)
        nc.gpsimd.memset(v4[:], 0.0)
        nc.gpsimd.memset(v4[:, :, :, 0:1], 1.0)
        make_identity(nc, ident[:])
        nc.gpsimd.partition_broadcast(vbb[:], vb[:])

        # ---- pools
        esb = ctx.enter_context(tc.tile_pool(name="esb", bufs=ESB_BUFS))
        netp = ctx.enter_context(tc.tile_pool(name="netp", bufs=4))
        recp = ctx.enter_context(tc.tile_pool(name="recp", bufs=4))
        osbp = ctx.enter_context(tc.tile_pool(name="osbp", bufs=6))
        rbbp = ctx.enter_context(tc.tile_pool(name="rbbp", bufs=2))
        sqp = ctx.enter_context(tc.tile_pool(name="sqp", bufs=3))

        dmyps = ctx.enter_context(tc.tile_pool(name="dmy", bufs=1, space="PSUM"))
        ph1 = ExitStack()
        wkps = ph1.enter_context(tc.tile_pool(name="work", bufs=3, space="PSUM"))
        vtps = ph1.enter_context(tc.tile_pool(name="vt", bufs=2, space="PSUM"))
        vpps = ph1.enter_context(tc.tile_pool(name="vp", bufs=1, space="PSUM"))
        esps = ph1.enter_context(tc.tile_pool(name="esps", bufs=2, space="PSUM"))
        eesb = ctx.enter_context(tc.tile_pool(name="eesb", bufs=26))

        # ---------------- phase 1 emitters ----------------
        def emit_q(ci, m):
            """q projection for token chunk ci (tap t, half), head pair m."""
            t, half = ci // 2, ci % 2
            cs = slice(ci * CHUNK, (ci + 1) * CHUNK)
            hs = slice(half * 512, (half + 1) * 512)
            qps = wkps.tile([P, CHUNK], F32, tag="wk")
            for u in range(2):
                for wi, wq in enumerate((qw8, ql8)):
                    nc.tensor.matmul(
                        qps[:], wq[:, u, :, m * P:(m + 1) * P],
                        xh8[:, 2 * u:2 * u + 2, t, hs],
                        start=(u == 0 and wi == 0),
                        stop=(u == 1 and wi == 1), perf_mode=DRM)
            if (ci + m) % 2 == 0:
                nc.scalar.activation(qT8[:, m, cs], qps[:],
                                     mybir.ActivationFunctionType.Identity,
                                     scale=1.0 / (SX * SQW))
            else:
                nc.vector.tensor_scalar_mul(qT8[:, m, cs], qps[:],
                                            1.0 / (SX * SQW))

        def emit_conv(n, after_oct=None):
            """conv for kv positions [n*512, (n+1)*512)."""
            ns = slice(n * 512, (n + 1) * 512)
            vt = vtps.tile([1, 512], F32, tag="vt")
            terms = [(wch, xh8), (wcl, xh8), (wch, xl8)]
            for oct_ in range(4):
                if after_oct is not None:
                    after_oct(oct_)
                cps = wkps.tile([P, 512], F32, tag="wk")
                first = True
                for wt, xt_ in terms:
                    for c in range(4):
                        for v in range(2):
                            w = c * 4 + 2 * v
                            nc.tensor.matmul(
                                cps[:],
                                wt[:, w:w + 2, oct_ * P:(oct_ + 1) * P],
                                xt_[:, c, 2 * v:2 * v + 2, ns],
                                start=first,
                                stop=(xt_ is xl8 and c == 3 and v == 1),
                                perf_mode=DRM)
                            first = False
                nc.scalar.activation(
                    xrc[:, oct_, ns], cps[:],
                    mybir.ActivationFunctionType.Identity,
                    bias=b4[:, oct_:oct_ + 1], scale=1.0 / (SX * SWC))
                sqt = sqp.tile([P, 512], BF, tag="sq")
                if SQ_POOL:
                    nc.gpsimd.tensor_tensor(
                        sqt[:], xrc[:, oct_, ns], xrc[:, oct_, ns],
                        mybir.AluOpType.mult)
                else:
                    nc.scalar.activation(
                        sqt[:], cps[:],
                        mybir.ActivationFunctionType.Square,
                        bias=b4[:, oct_:oct_ + 1], scale=1.0 / (SX * SWC))
                nc.tensor.matmul(vt[:], ones_inv[:], sqt[:],
                                 start=(oct_ == 0), stop=(oct_ == 3))
            return vt

        def emit_ln(n, vt):
            ns = slice(n * 512, (n + 1) * 512)
            nc.scalar.activation(
                stdt[0:1, ns], vt[:],
                mybir.ActivationFunctionType.Ln, bias=eps1[0:1, 0:1])
            nc.scalar.activation(rstd[0:1, ns], stdt[0:1, ns],
                                 mybir.ActivationFunctionType.Exp, scale=-0.5)
            rbb = rbbp.tile([P, 512], F32, tag="rbb")
            nc.gpsimd.partition_broadcast(rbb[:], rstd[0:1, ns])
            eng = nc.vector if XRN_DVE else nc.gpsimd
            for kt in range(4):
                eng.tensor_tensor(
                    xrn[:, kt, ns], xrc[:, kt, ns], rbb[:],
                    mybir.AluOpType.mult)

        def emit_k(n):
            ns = slice(n * 512, (n + 1) * 512)
            for m in range(2):
                kps = wkps.tile([P, 512], F32, tag="wk")
                for kt in range(4):
                    nc.tensor.matmul(
                        kps[:], kw[:, kt, m * P:(m + 1) * P], xrn[:, kt, ns],
                        start=(kt == 0), stop=False)
                # + bias as rank-1 outer product folded into the PSUM chain
                nc.tensor.matmul(
                    kps[:], kbr[0:1, m * P:(m + 1) * P], ones1[0:1, :],
                    start=False, stop=True, skip_group_check=True)
                nc.scalar.activation(
                    kT8[:, m, 0, ns], kps[:],
                    mybir.ActivationFunctionType.Identity, scale=SK)
                # fp8 residual into pair-1: scores DR sums (k_hi + k_lo).T @ q
                # (the broadcast rhs feeds q to both pairs), recovering the
                # fp8 storage rounding of k for free.
                nc.vector.scalar_tensor_tensor(
                    kT8[:, m, 1, ns], kps[:], SK,
                    kT8[:, m, 0, ns],
                    mybir.AluOpType.mult, mybir.AluOpType.subtract)

        def emit_v(n):
            for pt in range(4 * n, 4 * (n + 1)):
                vps = vpps.tile([P, 256], F32, tag="vp")
                for kt in range(4):
                    nc.tensor.matmul(
                        vps[:], xrn[:, kt, pt * P:(pt + 1) * P], vw[:, kt, :],
                        start=(kt == 0), stop=(kt == 3))
                nc.vector.scalar_tensor_tensor(
                    v4[:, pt, :, 1:65],
                    vps[:].rearrange("p (h d) -> p h d", h=4),
                    0.0,
                    vbb[:].rearrange("p (h d) -> p h d", h=4),
                    mybir.AluOpType.add,
                    mybir.AluOpType.add)

        # ---------------- phase 2 emitters ----------------
        state = {}

        def _qb(i):
            ci, h = i // 4, i % 4
            cs = slice(ci * CHUNK, (ci + 1) * CHUNK)
            po = 64 * (h % 2)
            return qT8[po:po + 64, h // 2, cs].rearrange(
                "p (one n) -> p one n", one=1).broadcast_to([64, 2, CHUNK])

        def emit_scores(i, js=(0, 1, 2, 3)):
            h = i % 4
            qb = _qb(i)
            etiles = state.setdefault(i, [])
            if SPS1:
                for j in js:
                    for ti in range(2):
                        t = 2 * j + ti
                        sps = spsp.tile([P, 1, CHUNK], F32, tag="s")
                        nc.tensor.matmul(
                            sps[:, 0, :],
                            kT8[0:64, h, :, t * P:(t + 1) * P],
                            qb, start=True, stop=True, perf_mode=DRM)
                        ebf = esb.tile([P, 1, CHUNK], BF, tag="e")
                        if _exp_on_dve(i, 2 * j + ti):
                            nc.vector._custom_dve(
                                EXP8, out=ebf[:], in0=sps[:],
                                s0=EXP8_C[0], s1=EXP8_C[1], imm2=EXP8_C[2])
                        else:
                            nc.scalar.activation(
                                ebf[:], sps[:],
                                mybir.ActivationFunctionType.Exp, scale=SCALE / SK)
                        etiles.append((ebf, 0))
                return
            for j in js:
                if j == 2 and DUMMY_SC:
                    emit_dummy(DUMMY_SC)
                sps = spsp.tile([P, 2, CHUNK], F32, tag="s")
                po = 64 * (h % 2)
                for ti in range(2):
                    t = 2 * j + ti
                    nc.tensor.matmul(
                        sps[:, ti, :],
                        kT8[po:po + 64, h // 2, :, t * P:(t + 1) * P],
                        qb, start=True, stop=True, perf_mode=DRM)
                ebf = esb.tile([P, 2, CHUNK], BF, tag="e")
                if _exp_on_dve(i, j):
                    nc.vector._custom_dve(
                        EXP8, out=ebf[:], in0=sps[:],
                        s0=EXP8_C[0], s1=EXP8_C[1], imm2=EXP8_C[2])
                else:
                    nc.scalar.activation(
                        ebf[:], sps[:],
                        mybir.ActivationFunctionType.Exp, scale=SCALE / SK)
                etiles.append((ebf, 0))
                etiles.append((ebf, 1))

        def emit_scores_early(i):
            """Chunks 0-3 of task i as single-chunk tiles (phase-1 PSUM)."""
            h = i % 4
            qb = _qb(i)
            etiles = state.setdefault(i, [])
            for t in range(4):
                sps = esps.tile([P, 1, CHUNK], F32, tag="es")
                po = 64 * (h % 2)
                nc.tensor.matmul(
                    sps[:, 0, :],
                    kT8[po:po + 64, h // 2, :, t * P:(t + 1) * P],
                    qb, start=True, stop=True, perf_mode=DRM)
                ebf = eesb.tile([P, 1, CHUNK], BF, tag="ee")
                if t % 2 == 1:
                    nc.vector._custom_dve(
                        EXP8, out=ebf[:], in0=sps[:],
                        s0=EXP8_C[0], s1=EXP8_C[1], imm2=EXP8_C[2])
                else:
                    nc.scalar.activation(
                        ebf[:], sps[:],
                        mybir.ActivationFunctionType.Exp, scale=SCALE / SK)
                etiles.append((ebf, 0))

        def emit_dummy(n_mm):
            """PE keep-warm: junk matmuls into a sacrificial PSUM bank so the
            tensor engine's p-state ramp survives element-engine waits."""
            dt_ = dmyps.tile([P, CHUNK], F32, tag="dm")
            rhs = ident[:].rearrange("p (o n) -> p o n", o=1).broadcast_to(
                [P, 4, P])
            for d in range(n_mm):
                nc.tensor.matmul(dt_[:], ident[:], rhs, start=True, stop=True)

        def emit_av_head(i):
            h = i % 4
            etiles = state.pop(i)
            avp = avps.tile([P, 4, 65], F32, tag="av")
            # t-outer: the first matmuls only need etile[0], so av starts
            # while the later exp instructions are still running.
            if AV_TOUTER:
                order = [(t, s) for t in range(8) for s in range(4)]
            else:
                order = [(t, s) for s in range(4) for t in range(8)]
            for t, s in order:
                ebf, sub = etiles[t]
                nc.tensor.matmul(
                    avp[:, s, :],
                    ebf[:, sub, s * P:(s + 1) * P],
                    v4[:, t, h, :],
                    start=(t == 0), stop=(t == 7))
            rec = recp.tile([P, 4, 1], F32, tag="rec")
            nc.vector.reciprocal_approx_fast(out=rec[:], in_=avp[:, :, 0:1])
            net = netp.tile([P, 4, 64], BF, tag="net")
            nc.vector.tensor_tensor(
                net[:], avp[:, :, 1:65], rec[:].broadcast_to([P, 4, 64]),
                mybir.AluOpType.mult)
            state[("net", i)] = net

        def emit_av_tail(i):
            ci, h = i // 4, i % 4
            cs = slice(ci * CHUNK, (ci + 1) * CHUNK)
            net = state.pop(("net", i))
            ntp = ntpp.tile([64, 4, P], BF, tag="nt")
            for s in range(4):
                nc.tensor.transpose(ntp[:, s, :], net[:, s, :], ident[:])
            po = 64 * (h % 2)
            nc.vector.tensor_copy(
                netT[po:po + 64, h // 2, cs].rearrange(
                    "p (s n) -> p s n", s=4),
                ntp[:])

        def emit_proj_oct(ci, oct_):
            cs = slice(ci * CHUNK, (ci + 1) * CHUNK)
            pps = prps.tile([P, CHUNK], F32, tag="pr")
            for kt in range(2):
                nc.tensor.matmul(
                    pps[:], pw[:, kt, oct_ * P:(oct_ + 1) * P],
                    netT[:, kt, cs],
                    start=(kt == 0), stop=(kt == 1))
            osb = osbp.tile([P, CHUNK], BF, tag="osb")
            nc.scalar.activation(
                osb[:], pps[:],
                mybir.ActivationFunctionType.Identity)
            nc.sync.dma_start(out_d[oct_ * P:(oct_ + 1) * P, cs], osb[:])

        # ---------------- emission schedule ----------------
        W = int(_os.environ.get("K_W", "7"))
        if DUMMY_WARM:
            emit_dummy(DUMMY_WARM)  # warm the PE ramp during DMA wait
        nq_pre = int(_os.environ.get("K_NQ_PRE", "8"))
        for ci in range(8):
            for m in range(2):
                if ci < nq_pre:
                    emit_q(ci, m)
        vt0 = emit_conv(0)
        emit_ln(0, vt0)
        for ci in range(nq_pre, 8):
            for m in range(2):
                emit_q(ci, m)
        vt1 = emit_conv(1)
        emit_k(0)
        emit_v(0)
        emit_ln(1, vt1)
        # early half-scores (kv chunks 0-3) for the first W tasks: their
        # exps keep ACT/DVE busy while PE finishes k1/v1.
        if W:
            emit_scores_early(0)
            emit_scores_early(1)
        emit_k(1)
        for i in range(2, W):
            if i == 4:
                emit_v(1)
            emit_scores_early(i)
        if W <= 4:
            emit_v(1)
        ph1.close()

        ph2 = ExitStack()
        spsp = ph2.enter_context(tc.tile_pool(
            name="sps", bufs=(5 if SPS1 else 2), space="PSUM"))
        avps = ph2.enter_context(tc.tile_pool(name="avp", bufs=1, space="PSUM"))
        ntpp = ph2.enter_context(tc.tile_pool(
            name="ntp", bufs=(1 if SPS1 else 2), space="PSUM"))
        prps = ph2.enter_context(tc.tile_pool(name="prj", bufs=1, space="PSUM"))
        NT = 32
        projq = []
        for i in range(NT + 2):
            if i < NT:
                emit_scores(i, (2, 3) if i < W else (0, 1, 2, 3))
            if 1 <= i <= NT:
                if DUMMY_AV:
                    emit_dummy(DUMMY_AV)
                emit_av_head(i - 1)
            if 2 <= i <= NT + 1:
                emit_av_tail(i - 2)
                if (i - 2) % 4 == 3:
                    projq += [((i - 2) // 4, o) for o in range(4)]
            if projq:
                emit_proj_oct(*projq.pop(0))
        while projq:
            emit_proj_oct(*projq.pop(0))
        ph2.close()

        if DEBUG:
            with tc.tile_pool(name="dbg", bufs=1) as dp:
                def dump(dram, sb_ap, nparts):
                    total = 1
                    for s in sb_ap.shape[1:]:
                        total *= s
                    pat = {2: None, 3: "p a b -> p (a b)",
                           4: "p a b c -> p (a b c)"}[len(sb_ap.shape)]
                    flat = sb_ap.rearrange(pat) if pat else sb_ap
                    dfl = dram.rearrange(pat) if pat else dram
                    for off in range(0, total, 512):
                        w_ = min(512, total - off)
                        tt = dp.tile([nparts, 512], F32, tag="dbgt")
                        nc.vector.tensor_copy(tt[:, :w_], flat[:, off:off + w_])
                        nc.sync.dma_start(dfl[:, off:off + w_], tt[:, :w_])
                dump(dbg["xrn"], xrn[:], P)
                dump(dbg["qT8"], qT8[:], P)
                dump(dbg["kT8"], kT8[:], P)
                dump(dbg["v4"], v4[:], P)
                dump(dbg["netT"], netT[:], P)

    nc.finalize()
    return nc


_NC_CACHE = {}


def _get_nc():
    if "nc" not in _NC_CACHE:
        _NC_CACHE["nc"] = build_nc()
    return _NC_CACHE["nc"]


def _token_perm():
    """tokp = t*1024 + p (t = 2di+dj, p = 32i+j) -> original token index."""
    di = np.array([0, 0, 1, 1])
    dj = np.array([0, 1, 0, 1])
    i, j = np.meshgrid(np.arange(32), np.arange(32), indexing="ij")
    perm = np.empty(4096, np.int64)
    for t in range(4):
        r = (2 * i + di[t]) * 64 + (2 * j + dj[t])
        perm[t * 1024:(t + 1) * 1024] = r.reshape(-1)
    return perm


TOKPERM = _token_perm()


def _prep_core_inputs(x, q_w, kv_w, proj_w, sr_w, sr_b, ln_w, ln_b):
    # conv weights: W4[t, ic, oc] = sr_w[oc, ic, di, dj], t = di*2+dj,
    # centered over oc so LN mean-subtraction is free.
    W4 = np.ascontiguousarray(sr_w.transpose(2, 3, 1, 0)).reshape(4, DIM, DIM)
    W4 = W4 - W4.mean(axis=2, keepdims=True)
    # wc[k = c*4 + t][p][oc] = W4[t, c*128+p, oc]
    wc = np.ascontiguousarray(
        W4.reshape(4, 4, P, DIM).transpose(1, 0, 2, 3)).reshape(16, P, DIM)
    wcs = wc * SWC
    wch = wcs.astype(NF8)
    wcl = (wcs - wch.astype(np.float32)).astype(NF8)
    b4 = (sr_b - sr_b.mean()).reshape(4, P).T.astype(np.float32)
    b4 = np.ascontiguousarray(b4)

    kv_w_f = ln_w[:, None] * kv_w
    kv_bias = ln_b @ kv_w

    in_maps = []
    for core in range(8):
        b, g = core // 2, core % 2
        xT = np.ascontiguousarray(x[b].T[:, TOKPERM])      # (512, 4096) permuted
        xs = xT.reshape(4, P, 4, NKV) * SX
        xh8 = xs.astype(NF8)
        xl8 = (xs - xh8.astype(np.float32)).astype(NF8)
        qsl = q_w[:, g * 256:(g + 1) * 256]
        # qw8[p, u, cc, col] = q_w[(2u+cc)*128 + p, g*256+col]
        qarr = np.ascontiguousarray(
            qsl.reshape(2, 2, P, 256).transpose(2, 0, 1, 3)) * SQW
        qw8 = qarr.astype(NF8)
        ql8 = (qarr - qw8.astype(np.float32)).astype(NF8)
        kw = np.ascontiguousarray(
            kv_w_f[:, g * 256:(g + 1) * 256]).reshape(4, P, 256).astype(NBF)
        vw = np.ascontiguousarray(
            kv_w_f[:, DIM + g * 256:DIM + (g + 1) * 256]
        ).reshape(4, P, 256).astype(NBF)
        pwv = np.ascontiguousarray(
            proj_w[g * 256:(g + 1) * 256, :]).reshape(2, P, DIM).astype(NBF)
        kbv = kv_bias[g * 256:(g + 1) * 256].reshape(1, 256).astype(NBF)
        vbv = kv_bias[DIM + g * 256:DIM + (g + 1) * 256].reshape(1, 256)
        in_maps.append({
            "xh8": xh8, "xl8": xl8, "wch": wch, "wcl": wcl, "qw": qw8,
            "ql": ql8, "kw": kw, "vw": vw,
            "pw": pwv, "b4": b4, "kbr": kbv, "vb": vbv.astype(NBF),
        })
    return in_maps


def kernel(x, q_w, kv_w, proj_w, proj_b, sr_w, sr_b, ln_w, ln_b, H, W,
           _return_perf=False):
    x = np.asarray(x, dtype=np.float32)
    q_w = np.asarray(q_w, dtype=np.float32)
    kv_w = np.asarray(kv_w, dtype=np.float32)
    proj_w = np.asarray(proj_w, dtype=np.float32)
    proj_b = np.asarray(proj_b, dtype=np.float32)
    sr_w = np.asarray(sr_w, dtype=np.float32)
    sr_b = np.asarray(sr_b, dtype=np.float32)
    ln_w = np.asarray(ln_w, dtype=np.float32)
    ln_b = np.asarray(ln_b, dtype=np.float32)

    in_maps = _prep_core_inputs(x, q_w, kv_w, proj_w, sr_w, sr_b, ln_w, ln_b)
    nc = _get_nc()
    res = run_bass_kernel_spmd(nc, in_maps, core_ids=list(range(8)),
                               trace=_return_perf)
    out = np.empty((BS, N, DIM), dtype=np.float32)
    for b in range(BS):
        partial = (res.results[2 * b]["out"].astype(np.float32)
                   + res.results[2 * b + 1]["out"].astype(np.float32))
        out[b][TOKPERM, :] = partial.T
        out[b] += proj_b[None, :]
    if _return_perf:
        return out, res
    return out
